# revision 25
# baseline (speedup 1.0000x reference)
"""Trainium2 Bass kernel for nn_ContMlpPerFeature.

Computes, per feature f (32 of them):
    h = relu(r_[:, f, :] @ W1[f] + b1[f])     # [B, 128]
    y = relu(h @ W2[f] + b2[f])               # [B, 1]
    out[:, f, 0] = X[:, f]; out[:, f, 1] = y

Sharding: pure data-parallel over batch (B=16384) across 8 cores.

Design notes (v3):
  - X never touches the device: out[...,0] is X verbatim, so the host
    interleaves it during the unshard step.  The device computes y only.
  - Host-side prep (part of sharding): r_ sliced per core, transposed to
    [F, D, Bl] bf16; W1 packed to [128, F/2, H] (two features per 128
    partitions, matching the row-tiled L1 matmuls); b1 transposed to
    [H, F] f32; W2 to [H, F] bf16; b2 replicated to a [1, NBT*F] bf16
    row (plus a ones row) for the PSUM-init matmul.
  - Feature-pair-outer loop: for each pair, both 1024-column batch
    chunks are processed before moving on, so rT tile consumption
    (~2.2us/pair) never outruns DMA arrival (~1.5us/pair).
  - L1: per (feature, chunk), two N=512 bf16 matmuls (row-tiled,
    tile_position (0,0)/(64,0)) -> h_ps [128=H, 1024] fp32 (2 banks).
  - h drain: fused bias+relu PSUM->SBUF bf16, split across ScalarE and
    VectorE ~34:30 (their cycle times differ).  This drain is the
    throughput bottleneck: only ACT/DVE can read PSUM.
  - L2 uses h as the *stationary* operand: per (f, batch-tile-of-128),
    ldweights(h_f tile) + a single-column matmul with rhs=W2[:,f] writes
    y^T[128 batch, 1] directly into a single whole-batch yT PSUM bank.
    This replaces full-batch streaming L2 + PE transposes (~34us of PE
    time) with weight loads, and yields y batch-major for free.
  - b2 is pre-loaded into yT by a K=1 matmul (ones[1,128] x b2row,
    start=True); the per-column L2 matmuls accumulate onto it
    (start=False).  The y drain is then a pure relu + one DMA.
  - All DMAs use >=512B contiguous descriptors (host pre-packing) to
    avoid the small-element bandwidth penalty.
"""

from collections import deque

import ml_dtypes
import numpy as np

import concourse.bass as bass
import concourse.tile as tile
from concourse import bacc, mybir
from concourse.bass_utils import run_bass_kernel_spmd

F32 = mybir.dt.float32
BF16 = mybir.dt.bfloat16

N_CORES = 8
B_FULL, F, D = 16384, 32, 64
H = 2 * D  # 128
CHUNK = 1024

# Of the 2*F (feature, chunk) drain tiles, ScalarE takes 34 and VectorE 30
# (ACT is a bit faster per column).  In steady state the two engines
# alternate strictly (uneven interleaves stall the slower engine); ACT's 4
# extra tiles are taken during the first 16 drains, where the pipeline is
# paced by rT DMA arrival and both engines have idle slack anyway.
_EARLY_DVE = frozenset({1, 2, 3, 5, 7, 9, 11, 13, 15})


def _drain_on_act(i: int) -> bool:
    if i < 16:
        return i not in _EARLY_DVE
    return i % 2 == 0


def _bias_relu(nc, out_sb, in_ps, bias_ap, use_act):
    """Fused bias+relu PSUM->SBUF on ScalarE (use_act) or VectorE."""
    if use_act:
        nc.scalar.activation(
            out_sb, in_ps, mybir.ActivationFunctionType.Relu, bias=bias_ap
        )
    else:
        nc.vector.tensor_scalar(
            out=out_sb,
            in0=in_ps,
            scalar1=bias_ap,
            scalar2=0.0,
            op0=mybir.AluOpType.add,
            op1=mybir.AluOpType.max,
        )


def _build_nc(Bl: int) -> bass.Bass:
    assert Bl % CHUNK == 0
    n_chunks = Bl // CHUNK  # 2
    n_pairs = F // 2  # 16
    nbt = Bl // 128  # batch tiles (16)

    nc = bacc.Bacc()

    rT = nc.dram_tensor("rT", [F, D, Bl], BF16, kind="ExternalInput")
    w1p = nc.dram_tensor("w1p", [128, n_pairs, H], BF16, kind="ExternalInput")
    b1T = nc.dram_tensor("b1T", [H, F], F32, kind="ExternalInput")
    w2T = nc.dram_tensor("w2T", [H, F], BF16, kind="ExternalInput")
    # cb[0, 0:nbt*F] = b2 replicated per batch-tile; cb[0, nbt*F:+H] = ones
    cb = nc.dram_tensor("cb", [1, nbt * F + H], BF16, kind="ExternalInput")
    # yp[half, p, bt, fh]: half 0 = features 0..15, half 1 = features 16..31
    yp = nc.dram_tensor("yp", [2, 128, nbt, F // 2], F32, kind="ExternalOutput")

    with tile.TileContext(nc) as tc:
        with (
            tc.tile_pool(name="singles", bufs=1) as singles,
            tc.tile_pool(name="rpt", bufs=1) as p_rpt,
            tc.tile_pool(name="h", bufs=1) as p_h,
            tc.tile_pool(name="y", bufs=1) as p_y,
            tc.tile_pool(name="hps", bufs=3, space="PSUM") as p_hps,
            tc.tile_pool(name="yps", bufs=1, space="PSUM") as p_yps,
        ):
            # ---- input loads (order tuned so the serial DMA pipe feeds the
            # pipeline just-in-time: tiny first-pair weights, fine-grained
            # rpt for pairs 0-1, then bulk whole-pair transfers) -----------
            rt_vc = rT[:].rearrange(
                "(fp two) d (c b) -> fp c (two d) b", two=2, c=n_chunks
            )
            rt_vp = rT[:].rearrange("(fp two) d b -> fp (two d) b", two=2)
            # rpt_views[p][c] = (tile, column offset of chunk c)
            rpt_views = []
            for p in range(2):
                row = []
                for c in range(n_chunks):
                    rpt_pc = p_rpt.tile(
                        [128, CHUNK], BF16, bufs=1, tag=f"rpt{p}_{c}",
                        name=f"rpt{p}_{c}",
                    )
                    row.append((rpt_pc, 0))
                rpt_views.append(row)
            for p in range(2, n_pairs):
                rpt_p = p_rpt.tile(
                    [128, Bl], BF16, bufs=1, tag=f"rpt{p}", name=f"rpt{p}"
                )
                rpt_views.append([(rpt_p, c * CHUNK) for c in range(n_chunks)])

            w1p_t = singles.tile([128, n_pairs, H], BF16)
            nc.sync.dma_start(out=w1p_t[:, 0:2, :], in_=w1p[:, 0:2, :])
            nc.sync.dma_start(out=rpt_views[0][0][0], in_=rt_vc[0, 0])
            b1T_t = singles.tile([128, F], F32)
            nc.sync.dma_start(out=b1T_t, in_=b1T[:])
            nc.sync.dma_start(out=rpt_views[0][1][0], in_=rt_vc[0, 1])
            nc.sync.dma_start(out=rpt_views[1][0][0], in_=rt_vc[1, 0])
            nc.sync.dma_start(out=rpt_views[1][1][0], in_=rt_vc[1, 1])
            w2T_t = singles.tile([128, F], BF16)
            nc.sync.dma_start(out=w2T_t, in_=w2T[:])
            cb_t = singles.tile([1, nbt * F + H], BF16)
            nc.sync.dma_start(out=cb_t, in_=cb[:])
            nc.sync.dma_start(out=w1p_t[:, 2:, :], in_=w1p[:, 2:, :])
            for p in range(2, n_pairs):
                nc.sync.dma_start(out=rpt_views[p][0][0], in_=rt_vp[p])

            b2row = cb_t[:, 0 : nbt * F]
            ones = cb_t[:, nbt * F : nbt * F + H]

            # PE p-state warm-up: dummy matmuls on a zeroed tile keep the PE
            # continuously busy until the first rpt tile lands, so the ramp
            # clock (full speed ~3us after the busy epoch starts) has run
            # out by then and every real matmul goes at full rate.  Uses
            # the spare 8th PSUM bank.
            warm_sb = singles.tile([128, 512], BF16)
            nc.gpsimd.memset(warm_sb, 0.0)
            warm_ps = p_yps.tile([128, 512], F32, tag="warm")
            for _ in range(6):
                nc.tensor.matmul(
                    warm_ps, lhsT=warm_sb[:, 0:128], rhs=warm_sb,
                    start=True, stop=True,
                )

            # preload the ACT Relu table (hidden behind the input DMAs)
            act_warm = singles.tile([128, 2], F32)
            nc.gpsimd.memset(act_warm[:, 0:1], 0.0)
            nc.scalar.activation(
                act_warm[:, 1:2], act_warm[:, 0:1],
                mybir.ActivationFunctionType.Relu,
            )

            # single whole-batch yT accumulator: col = bt*F + f.  Its b2
            # pre-load matmul is emitted lazily (first emit_l2 call) so its
            # wait on the cb DMA never blocks the L1 stream.
            yT = p_yps.tile([128, nbt * F], F32, tag="yT")

            # ---- main pipeline -------------------------------------------
            # L2 for a (f, c) half-feature is emitted DELTA drains after its
            # L1 so the PE never waits on an unfinished drain at its queue
            # head.  Each pending item covers one chunk (8 batch tiles).
            DELTA = 4
            bt_per_chunk = CHUNK // 128  # 8
            pending = deque()  # (f, c, h_sb)
            yT_v = yT.rearrange("p (g f) -> p g f", f=F)

            def emit_half_out(half):
                # features half*16..half*16+15 are final: relu + DMA out
                y_sb = p_y.tile(
                    [128, nbt * F // 2], F32, bufs=1, tag=f"y{half}",
                    name=f"y_sb{half}",
                )
                # on ACT: it has slack both mid-steady-state and at the tail
                nc.scalar.activation(
                    y_sb,
                    yT_v[:, :, half * (F // 2) : (half + 1) * (F // 2)],
                    mybir.ActivationFunctionType.Relu,
                )
                nc.sync.dma_start(out=yp[half], in_=y_sb)

            yt_inited = False

            def emit_l2():
                nonlocal yt_inited
                if not yt_inited:
                    nc.tensor.matmul(
                        yT, lhsT=ones, rhs=b2row, start=True, stop=False,
                        skip_group_check=True,
                    )
                    yt_inited = True
                f, c, h_sb = pending.popleft()
                for bt in range(bt_per_chunk):
                    g = c * bt_per_chunk + bt  # global batch tile
                    nc.tensor.matmul(
                        yT[:, g * F + f : g * F + f + 1],
                        lhsT=h_sb[:, 128 * bt : 128 * (bt + 1)],
                        rhs=w2T_t[:, f : f + 1],
                        start=False,
                        stop=True,
                        skip_group_check=True,
                    )
                if f == F // 2 - 1 and c == n_chunks - 1:
                    emit_half_out(0)

            di = 0  # drain index
            for p in range(n_pairs):
                for c in range(n_chunks):
                    rpt, off = rpt_views[p][c]
                    for m in range(2):
                        f = 2 * p + m
                        h_ps = p_hps.tile([128, CHUNK], F32)
                        for s in range(CHUNK // 512):
                            nc.tensor.matmul(
                                h_ps[:, 512 * s : 512 * (s + 1)],
                                lhsT=w1p_t[64 * m : 64 * (m + 1), p, :],
                                rhs=rpt[
                                    64 * m : 64 * (m + 1),
                                    off + 512 * s : off + 512 * (s + 1),
                                ],
                                start=True,
                                stop=True,
                                tile_position=(64 * m, 0),
                            )
                        use_act = _drain_on_act(di)
                        di += 1
                        h_sb = p_h.tile(
                            [128, CHUNK],
                            BF16,
                            bufs=4,
                            tag="h_act" if use_act else "h_dve",
                        )
                        _bias_relu(nc, h_sb, h_ps, b1T_t[:, f : f + 1], use_act)
                        pending.append((f, c, h_sb))
                        if len(pending) > DELTA:
                            emit_l2()
            while pending:
                emit_l2()
            emit_half_out(1)

    nc.compile()
    return nc


_NC_CACHE: dict[int, bass.Bass] = {}


def _get_nc(Bl: int) -> bass.Bass:
    if Bl not in _NC_CACHE:
        _NC_CACHE[Bl] = _build_nc(Bl)
    return _NC_CACHE[Bl]


def _host_pack_weights(W1, b1, W2, b2, Bl):
    """Shared (replicated) device inputs, pre-packed for large-descriptor
    DMAs and the kernel's on-chip layouts."""
    nbt = Bl // 128
    W1bf = np.asarray(W1, dtype=np.float32).astype(ml_dtypes.bfloat16)
    # [F, D, H] -> [128=(m,d), F/2, H]: partitions 0:64 = W1[2p], 64:128 = W1[2p+1]
    w1p = np.ascontiguousarray(
        W1bf.reshape(F // 2, 2, D, H).transpose(1, 2, 0, 3).reshape(128, F // 2, H)
    )
    b1T = np.ascontiguousarray(np.asarray(b1, dtype=np.float32).T)  # [H, F]
    w2T = np.ascontiguousarray(
        np.asarray(W2, dtype=np.float32).reshape(F, H).T.astype(ml_dtypes.bfloat16)
    )  # [H, F]
    b2f = np.asarray(b2, dtype=np.float32).reshape(F)
    cb = np.zeros((1, nbt * F + H), dtype=ml_dtypes.bfloat16)
    cb[0, : nbt * F] = np.tile(b2f, nbt).astype(ml_dtypes.bfloat16)
    cb[0, nbt * F :] = np.float32(1.0)
    return w1p, b1T, w2T, cb


def _run(X, r_, W1, b1, W2, b2, trace=False, **spmd_kwargs):
    X = np.ascontiguousarray(np.asarray(X, dtype=np.float32))
    r_ = np.asarray(r_, dtype=np.float32)

    Btot = X.shape[0]
    assert Btot % N_CORES == 0
    Bl = Btot // N_CORES
    w1p, b1T, w2T, cb = _host_pack_weights(W1, b1, W2, b2, Bl)
    nc = _get_nc(Bl)

    in_maps = []
    for i in range(N_CORES):
        sl = slice(i * Bl, (i + 1) * Bl)
        # transpose + cast is part of host-side sharding: [Bl,F,D] -> [F,D,Bl]
        rT = np.ascontiguousarray(
            r_[sl].transpose(1, 2, 0).astype(ml_dtypes.bfloat16)
        )
        in_maps.append({"rT": rT, "w1p": w1p, "b1T": b1T, "w2T": w2T, "cb": cb})
    res = run_bass_kernel_spmd(
        nc, in_maps, core_ids=list(range(N_CORES)), trace=trace, **spmd_kwargs
    )
    # unshard: out[...,0] = X (host-side), out[...,1] = y from device
    out = np.empty((Btot, F, 2), dtype=np.float32)
    out[:, :, 0] = X
    for i in range(N_CORES):
        ypk = res.results[i]["yp"]  # [2, 128, nbt, F/2]; b = bt*128 + p
        y = np.concatenate([ypk[0], ypk[1]], axis=-1)  # [128, nbt, F]
        out[i * Bl : (i + 1) * Bl, :, 1] = y.transpose(1, 0, 2).reshape(Bl, F)
    return out, res


def kernel(X, r_, W1, b1, W2, b2):
    out, _ = _run(X, r_, W1, b1, W2, b2)
    return out
